# revision 12
# baseline (speedup 1.0000x reference)
"""Trainium2 Bass kernel for the LocalConnectivity diamond-ring stencil.

out[b, x, y] = sum_{1<=|dx|+|dy|<=5} w[|dx|+|dy|-1] * in[b, (x+dx)%512, (y+dy)%512]

Strategy (v5: pair-folded columns + host-packed fp16 I/O)
---------------------------------------------------------
Data-parallel over batch: 64 samples -> 8 cores x 8 samples. Per sample the
512x512 grid is processed in 5 row-tiles of 103 output rows.

Key identity: the dy=+k and dy=-k stencil columns share the same vertical
band weights, so with av_k[r, y] = x[r, y-k] + x[r, y+k] (horizontal shifts
are free AP offsets) the two dy=+-k PE passes collapse into ONE banded
matmul  psum += Band_k @ av_k.  Per tile the PE runs 6 matmuls (vs 9 for
the direct scheme):
  - j=0: dy=0 band on x itself (taps w1..w5 both sides, no shift)
  - j=k (k=1..4): band w_{|dx|+k}, |dx| <= 5-k, applied to av_k
  - j=5: diagonal w5 applied to av_5
Band is ALIGNED (out row 103t+p-5 at psum partition p), contraction 113.

Engine split:
  - TensorE: 6 PSUM-accumulating matmuls/tile, ~217 ns each warm.
  - VectorE: av builds, fp16 tensor_tensor in SBUF with unit stride and
    4B-aligned reads -> 2x_1P DVE mode. Even k needs odd offsets, so those
    builds read at (4-k, 4+k) [both even] and the matmul moving AP reads
    av_k at +1 element (moving APs have no alignment constraint).
  - ScalarE: psum -> SBUF fp16 eviction copies only.
  - Per-sample mode mix ('g1' 6 passes/5 builds vs 'direct1' 7 passes/4
    builds) balances cumulative PE vs DVE drift.

I/O (the v4 lesson: SDMA descriptors are latency-bound, ~11.6 GB/s per
engine on 2KB runs; 16 engines stripe one queue):
  - The HOST pre-gathers the input into fp16 [128, B, NT, 522]: partition
    p of tile t holds input row (103t+p-5)%512 with 5 circular halo
    columns on each side baked in. DRAM runs are 5.2KB/sample-partition,
    bytes are halved (no f32 read, no DMA cast), and no on-chip halo
    copies exist at all.
  - Output is written fp16 to [128, B, NT, 512] (5.1KB runs); the host
    un-gathers and casts to f32. Output SBUF tiles are already fp16.
  - Input DMAs are per-sample (plus a tiny sample0-tile0 DMA so the PE
    starts at ~9us); weight DMA first. All issued upfront on the Q7.
  - PE HAM clock gate: cold = 1.2 GHz, warm = 2.4 GHz after ~3.4us of
    sustained activity. The framework preamble occupies the first ~6.9us;
    4 dummy matmuls bridge to first-data-ready; sample 0's tile 0 runs
    the 11-pass direct scheme (zero DVE/weight-free deps) to keep the PE
    streaming while it warms.
"""

import numpy as np

import concourse.bass as bass
import concourse.bacc as bacc
import concourse.mybir as mybir
from concourse import tile
from concourse.bass_utils import run_bass_kernel_spmd

B, H, W = 64, 512, 512
NCORES = 8
BPC = B // NCORES  # samples per core
MAXD = 5
HALO = MAXD
TR = 103  # output rows per tile
NT = 5
CTR = TR + 2 * HALO  # 113 contraction rows
XW = W + 2 * HALO  # 522
NJ = 6  # stationaries: dy0 band + G1..G4 bands + G5 diag
AVW = 514  # av tile width (512 + 2 pad for the even-k alignment trick)

GROUPS = [(0, 2), (2, 3), (5, 3)]  # (b0, size) xt tiles
N_DUMMY = 7

F16 = mybir.dt.float16

# av-build read offsets: k odd reads at (5-k, 5+k) [even, even], moving
# offset 0; k even reads at (4-k, 4+k) [both even], moving offset 1.
AV_SPEC = {}
for _k in range(1, MAXD + 1):
    if _k % 2 == 1:
        AV_SPEC[_k] = (5 - _k, 5 + _k, 512, 0)  # (inA, inB, width, mov_off)
    else:
        AV_SPEC[_k] = (4 - _k, 4 + _k, 514, 1)


def _build_band_weights(dw: np.ndarray) -> np.ndarray:
    """[128, 6*128] fp16: stationary j at cols [128j, 128j+128).

    j=0: dy=0 band  B[p+dx, p] = w_{|dx|},   1 <= |dx| <= 5
    j=k: pair band  B[p+dx, p] = w_{|dx|+k}, |dx| <= 5-k   (k = 1..4)
    j=5: diagonal   B[p, p]    = w_5
    """
    wb = np.zeros((128, NJ, 128), dtype=np.float32)
    p = np.arange(128)
    for j in range(NJ):
        if j == 5:
            wb[p, j, p] = dw[MAXD - 1]
            continue
        k = j
        for dx in range(-(MAXD - k), MAXD - k + 1):
            d = abs(dx) + k
            if not (1 <= d <= MAXD):
                continue
            c = p + dx
            valid = (c >= 0) & (c < 128)
            wb[c[valid], j, p[valid]] = dw[d - 1]
    return np.ascontiguousarray(wb.reshape(128, NJ * 128).astype(np.float16))


_CACHED_NC = None


def _custom_ap(base_ap, dims, extra_offset_elems=0):
    """Build a strided AP: dims = [(stride_elems, size), ...]."""
    s = base_ap.copy()
    s.ap.clear()
    s.ap.extend(dims)
    s.offset = s.offset + extra_offset_elems
    return s


def _build_program():
    f32 = mybir.dt.float32

    nc = bacc.Bacc(None, target_bir_lowering=False)
    # host-packed: x[p, b*NT*XW + t*XW + y], y[p, b*NT*W + t*W + y]
    x = nc.dram_tensor("x", [128, BPC * NT * XW], F16, kind="ExternalInput")
    wb = nc.dram_tensor("wb", [128, NJ * 128], F16, kind="ExternalInput")
    y = nc.dram_tensor("y", [128, BPC * NT * W], F16, kind="ExternalOutput")

    XROW = BPC * NT * XW  # dram row stride (elems per partition)
    YROW = BPC * NT * W

    with tile.TileContext(nc) as tc:
        with (
            tc.tile_pool(name="wpool", bufs=1) as wpool,
            tc.tile_pool(name="xpool_a", bufs=1) as xpool_a,
            tc.tile_pool(name="xpool_b", bufs=2) as xpool_b,
            tc.tile_pool(name="avpool", bufs=2) as avpool,
            tc.tile_pool(name="opool", bufs=3) as opool,
            tc.tile_pool(name="pspool", bufs=8, space=bass.MemorySpace.PSUM) as pspool,
        ):
            wtile = wpool.tile([128, NJ * 128], F16, tag="wt")

            # PE warm-up across the preamble->first-data window
            dummy = wpool.tile([128, 640], F16, tag="dummy")
            nc.vector.memset(dummy[:], 0.0)
            wpt = pspool.tile([128, W], f32, tag="pt")
            for _ in range(N_DUMMY):
                nc.tensor.matmul(wpt[0:TR, :], dummy[0:CTR, 0:TR],
                                 dummy[0:CTR, 64:576], start=True, stop=True)

            # ---- input DMAs: all issued upfront, per sample; sample 0's
            # tile 0 gets its own small first DMA so the PE can start
            # immediately after it lands ----
            sample_xt = {}
            for gi, (pool, (b0, gsz)) in enumerate(
                zip((xpool_a, xpool_b, xpool_b), GROUPS)
            ):
                xt = pool.tile([128, gsz, NT, XW], F16,
                               tag="xta" if gi == 0 else "xtbc")
                for bi in range(gsz):
                    b = b0 + bi
                    if b == 0:
                        src0 = _custom_ap(x[:], [(XROW, 128), (1, XW)])
                        nc.gpsimd.dma_start(xt[:, 0, 0, :], src0)
                        nc.gpsimd.dma_start(wtile[:], wb[:])
                        src1 = _custom_ap(
                            x[:], [(XROW, 128), (XW, NT - 1), (1, XW)],
                            extra_offset_elems=XW,
                        )
                        nc.gpsimd.dma_start(xt[:, 0, 1:NT, :], src1)
                    else:
                        src = _custom_ap(
                            x[:], [(XROW, 128), (1, NT * XW)],
                            extra_offset_elems=b * NT * XW,
                        )
                        nc.gpsimd.dma_start(
                            xt[:, bi, :, :], src
                        )
                    sample_xt[b] = (xt, bi)

            # Per-sample mode mix: 'g1' = 6 PE passes + 5 DVE builds,
            # 'direct1' = 7 passes + 4 builds. Sample 0 special-cased.
            MODES = [None, 'g1', 'g1', 'g1', 'g1', 'g1', 'g1', 'g1']

            def pass_list(mode):
                if mode == 'direct':
                    ps = [(0, ('x', HALO))]
                    for k in range(1, MAXD + 1):
                        ps += [(k, ('x', HALO - k)), (k, ('x', HALO + k))]
                elif mode == 'direct1':
                    ps = [(0, ('x', HALO)),
                          (1, ('x', HALO - 1)), (1, ('x', HALO + 1))]
                    ps += [(k, ('av', k)) for k in range(2, MAXD + 1)]
                else:
                    ps = [(0, ('x', HALO))]
                    ps += [(k, ('av', k)) for k in range(1, MAXD + 1)]
                return ps

            for b in range(BPC):
                xt, bq = sample_xt[b]
                mode = MODES[b]
                build_ks = (tuple(range(1, MAXD + 1)) if mode in (None, 'g1')
                            else tuple(range(2, MAXD + 1)))

                # ---- av builds on DVE (fp16 2x mode) ----
                avs = {}
                for k in build_ks:
                    avs[k] = avpool.tile([128, NT, AVW], F16, tag=f"av{k}",
                                         name=f"av{k}")
                if b == 0:
                    # tile 0 is computed direct; build only tiles 1-4
                    for k in build_ks:
                        a0, a1, wdt, _ = AV_SPEC[k]
                        nc.vector.tensor_add(
                            avs[k][0:CTR, 1:NT, 0:wdt],
                            xt[0:CTR, bq, 1:NT, a0 : a0 + wdt],
                            xt[0:CTR, bq, 1:NT, a1 : a1 + wdt],
                        )
                else:
                    for k in build_ks:
                        a0, a1, wdt, _ = AV_SPEC[k]
                        nc.vector.tensor_add(
                            avs[k][0:CTR, :, 0:wdt],
                            xt[0:CTR, bq, :, a0 : a0 + wdt],
                            xt[0:CTR, bq, :, a1 : a1 + wdt],
                        )

                pts = [pspool.tile([128, W], f32, tag="pt", name=f"pt{t}")
                       for t in range(NT)]

                def moving(src, t):
                    kind, v = src
                    if kind == 'x':
                        return xt[0:CTR, bq, t, v : v + W]
                    _, _, _, mo = AV_SPEC[v]
                    return avs[v][0:CTR, t, mo : mo + W]

                if b == 0:
                    tile_passes = {0: pass_list('direct')}
                    for t in range(1, NT):
                        tile_passes[t] = pass_list('g1')
                    loop = [(t, pi) for t in range(NT)
                            for pi in range(len(tile_passes[t]))]
                else:
                    ps = pass_list(mode)
                    tile_passes = {t: ps for t in range(NT)}
                    if b == BPC - 1:
                        # t-outer: staggers psum completion so eviction +
                        # output drain inside the stream
                        loop = [(t, pi) for t in range(NT)
                                for pi in range(len(ps))]
                    else:
                        loop = [(t, pi) for pi in range(len(ps))
                                for t in range(NT)]
                for t, pi in loop:
                    j, src = tile_passes[t][pi]
                    nc.tensor.matmul(
                        pts[t][0:108, :],
                        wtile[0:CTR, j * 128 : j * 128 + 108],
                        moving(src, t),
                        start=(pi == 0),
                        stop=(pi == len(tile_passes[t]) - 1),
                    )

                # ---- eviction on ScalarE: psum -> fp16 SBUF copy ----
                otb = opool.tile([128, NT, W], F16, tag="otb")
                for t in range(NT):
                    if b == BPC - 1 and t >= 3:
                        nc.vector.tensor_copy(otb[0:108, t, :],
                                              pts[t][0:108, :])
                    else:
                        nc.scalar.copy(otb[0:108, t, :], pts[t][0:108, :])

                # ---- output DMAs: fp16, 5.1KB runs per partition ----
                if b == BPC - 1:
                    splits = ((0, 1), (1, 2), (2, 3), (3, 4), (4, 5))
                elif b == BPC - 2:
                    splits = ((0, 2), (2, 4), (4, 5))
                else:
                    splits = ((0, NT),)
                for lo, hi in splits:
                    dstp = _custom_ap(
                        y[:], [(YROW, 128), (1, (hi - lo) * W)],
                        extra_offset_elems=(b * NT + lo) * W,
                    )
                    nc.gpsimd.dma_start(
                        dstp, otb[:, lo:hi, :]
                    )
    nc.compile()
    return nc


def _get_program():
    global _CACHED_NC
    if _CACHED_NC is None:
        _CACHED_NC = _build_program()
    return _CACHED_NC


def _pack_input(grid_spikes: np.ndarray) -> np.ndarray:
    """[B,H,W] f32 -> [128, B, NT, XW] fp16 with rows (103t+p-5)%512 and
    5 circular halo columns on each side."""
    p = np.arange(128)
    t = np.arange(NT)
    rows = (TR * t[:, None] + p[None, :] - HALO) % H  # [NT, 128]
    g = grid_spikes[:, rows, :]  # [B, NT, 128, W]
    g = np.concatenate([g[..., H - HALO :], g, g[..., :HALO]], axis=-1)
    return np.ascontiguousarray(
        g.transpose(2, 0, 1, 3).astype(np.float16)
    )  # [128, B, NT, XW]


def _run(grid_spikes, distance_weights, trace=False):
    grid_spikes = np.ascontiguousarray(np.asarray(grid_spikes, dtype=np.float32))
    distance_weights = np.asarray(distance_weights, dtype=np.float32)
    assert grid_spikes.shape == (B, H, W), grid_spikes.shape
    wb_np = _build_band_weights(distance_weights)
    xh = _pack_input(grid_spikes)  # [128, B, NT, XW] fp16

    nc = _get_program()
    in_maps = [
        {
            "x": np.ascontiguousarray(
                xh[:, i * BPC : (i + 1) * BPC]
            ).reshape(128, BPC * NT * XW),
            "wb": wb_np,
        }
        for i in range(NCORES)
    ]
    res = run_bass_kernel_spmd(nc, in_maps, list(range(NCORES)), trace=trace)
    yh = np.concatenate(
        [res.results[i]["y"].reshape(128, BPC, NT, W) for i in range(NCORES)],
        axis=1,
    )  # [128, B, NT, W] fp16
    # partition p of tile t holds out row 103t + p - 5 (valid p in 5..107);
    # rows 512..514 of the last tile are circular duplicates to drop.
    out = (
        yh[HALO : HALO + TR]
        .transpose(1, 2, 0, 3)
        .reshape(B, NT * TR, W)[:, :H, :]
        .astype(np.float32)
    )
    return np.ascontiguousarray(out), res


def kernel(grid_spikes, distance_weights):
    out, _ = _run(grid_spikes, distance_weights, trace=False)
    return out


def kernel_traced(grid_spikes, distance_weights):
    out, res = _run(grid_spikes, distance_weights, trace=True)
    return out, res


# revision 13
# speedup vs baseline: 1.0406x; 1.0406x over previous
"""Trainium2 Bass kernel for the LocalConnectivity diamond-ring stencil.

out[b, x, y] = sum_{1<=|dx|+|dy|<=5} w[|dx|+|dy|-1] * in[b, (x+dx)%512, (y+dy)%512]

Strategy (v5: pair-folded columns + host-packed fp16 I/O)
---------------------------------------------------------
Data-parallel over batch: 64 samples -> 8 cores x 8 samples. Per sample the
512x512 grid is processed in 5 row-tiles of 103 output rows.

Key identity: the dy=+k and dy=-k stencil columns share the same vertical
band weights, so with av_k[r, y] = x[r, y-k] + x[r, y+k] (horizontal shifts
are free AP offsets) the two dy=+-k PE passes collapse into ONE banded
matmul  psum += Band_k @ av_k.  Per tile the PE runs 6 matmuls (vs 9 for
the direct scheme):
  - j=0: dy=0 band on x itself (taps w1..w5 both sides, no shift)
  - j=k (k=1..4): band w_{|dx|+k}, |dx| <= 5-k, applied to av_k
  - j=5: diagonal w5 applied to av_5
Band is ALIGNED (out row 103t+p-5 at psum partition p), contraction 113.

Engine split:
  - TensorE: 6 PSUM-accumulating matmuls/tile, ~217 ns each warm.
  - VectorE: av builds, fp16 tensor_tensor in SBUF with unit stride and
    4B-aligned reads -> 2x_1P DVE mode. Even k needs odd offsets, so those
    builds read at (4-k, 4+k) [both even] and the matmul moving AP reads
    av_k at +1 element (moving APs have no alignment constraint).
  - ScalarE: psum -> SBUF fp16 eviction copies only.
  - Per-sample mode mix ('g1' 6 passes/5 builds vs 'direct1' 7 passes/4
    builds) balances cumulative PE vs DVE drift.

I/O (the v4 lesson: SDMA descriptors are latency-bound, ~11.6 GB/s per
engine on 2KB runs; 16 engines stripe one queue):
  - The HOST pre-gathers the input into fp16 [128, B, NT, 522]: partition
    p of tile t holds input row (103t+p-5)%512 with 5 circular halo
    columns on each side baked in. DRAM runs are 5.2KB/sample-partition,
    bytes are halved (no f32 read, no DMA cast), and no on-chip halo
    copies exist at all.
  - Output is written fp16 to [128, B, NT, 512] (5.1KB runs); the host
    un-gathers and casts to f32. Output SBUF tiles are already fp16.
  - Input DMAs are per-sample (plus a tiny sample0-tile0 DMA so the PE
    starts at ~9us); weight DMA first. All issued upfront on the Q7.
  - PE HAM clock gate: cold = 1.2 GHz, warm = 2.4 GHz after ~3.4us of
    sustained activity. The framework preamble occupies the first ~6.9us;
    4 dummy matmuls bridge to first-data-ready; sample 0's tile 0 runs
    the 11-pass direct scheme (zero DVE/weight-free deps) to keep the PE
    streaming while it warms.
"""

import numpy as np

import concourse.bass as bass
import concourse.bacc as bacc
import concourse.mybir as mybir
from concourse import tile
from concourse.bass_utils import run_bass_kernel_spmd

B, H, W = 64, 512, 512
NCORES = 8
BPC = B // NCORES  # samples per core
MAXD = 5
HALO = MAXD
TR = 103  # output rows per tile
NT = 5
CTR = TR + 2 * HALO  # 113 contraction rows
XW = W + 2 * HALO  # 522
NJ = 6  # stationaries: dy0 band + G1..G4 bands + G5 diag
AVW = 514  # av tile width (512 + 2 pad for the even-k alignment trick)

GROUPS = [(0, 2), (2, 3), (5, 3)]  # (b0, size) xt tiles
N_DUMMY = 6

F16 = mybir.dt.float16

# av-build read offsets: k odd reads at (5-k, 5+k) [even, even], moving
# offset 0; k even reads at (4-k, 4+k) [both even], moving offset 1.
AV_SPEC = {}
for _k in range(1, MAXD + 1):
    if _k % 2 == 1:
        AV_SPEC[_k] = (5 - _k, 5 + _k, 512, 0)  # (inA, inB, width, mov_off)
    else:
        AV_SPEC[_k] = (4 - _k, 4 + _k, 514, 1)


def _build_band_weights(dw: np.ndarray) -> np.ndarray:
    """[128, 6*128] fp16: stationary j at cols [128j, 128j+128).

    j=0: dy=0 band  B[p+dx, p] = w_{|dx|},   1 <= |dx| <= 5
    j=k: pair band  B[p+dx, p] = w_{|dx|+k}, |dx| <= 5-k   (k = 1..4)
    j=5: diagonal   B[p, p]    = w_5
    """
    wb = np.zeros((128, NJ, 128), dtype=np.float32)
    p = np.arange(128)
    for j in range(NJ):
        if j == 5:
            wb[p, j, p] = dw[MAXD - 1]
            continue
        k = j
        for dx in range(-(MAXD - k), MAXD - k + 1):
            d = abs(dx) + k
            if not (1 <= d <= MAXD):
                continue
            c = p + dx
            valid = (c >= 0) & (c < 128)
            wb[c[valid], j, p[valid]] = dw[d - 1]
    return np.ascontiguousarray(wb.reshape(128, NJ * 128).astype(np.float16))


_CACHED_NC = None


def _custom_ap(base_ap, dims, extra_offset_elems=0):
    """Build a strided AP: dims = [(stride_elems, size), ...]."""
    s = base_ap.copy()
    s.ap.clear()
    s.ap.extend(dims)
    s.offset = s.offset + extra_offset_elems
    return s


def _build_program():
    f32 = mybir.dt.float32

    nc = bacc.Bacc(None, target_bir_lowering=False)
    # host-packed: x[p, b*NT*XW + t*XW + y], y[p, b*NT*W + t*W + y]
    x = nc.dram_tensor("x", [128, BPC * NT * XW], F16, kind="ExternalInput")
    wb = nc.dram_tensor("wb", [128, NJ * 128], F16, kind="ExternalInput")
    y = nc.dram_tensor("y", [128, BPC * NT * W], F16, kind="ExternalOutput")

    XROW = BPC * NT * XW  # dram row stride (elems per partition)
    YROW = BPC * NT * W

    with tile.TileContext(nc) as tc:
        with (
            tc.tile_pool(name="wpool", bufs=1) as wpool,
            tc.tile_pool(name="xpool_a", bufs=1) as xpool_a,
            tc.tile_pool(name="xpool_b", bufs=2) as xpool_b,
            tc.tile_pool(name="avpool", bufs=2) as avpool,
            tc.tile_pool(name="opool", bufs=3) as opool,
            tc.tile_pool(name="pspool", bufs=8, space=bass.MemorySpace.PSUM) as pspool,
        ):
            wtile = wpool.tile([128, NJ * 128], F16, tag="wt")
            nc.gpsimd.dma_start(wtile[:], wb[:])

            # PE warm-up across the preamble->first-data window
            dummy = wpool.tile([128, 640], F16, tag="dummy")
            nc.vector.memset(dummy[:], 0.0)
            wpt = pspool.tile([128, W], f32, tag="pt")
            for _ in range(N_DUMMY):
                nc.tensor.matmul(wpt[0:TR, :], dummy[0:CTR, 0:TR],
                                 dummy[0:CTR, 64:576], start=True, stop=True)

            # ---- input DMAs: all issued upfront, per sample; sample 0's
            # tile 0 gets its own small first DMA so the PE can start
            # immediately after it lands ----
            sample_xt = {}
            for gi, (pool, (b0, gsz)) in enumerate(
                zip((xpool_a, xpool_b, xpool_b), GROUPS)
            ):
                xt = pool.tile([128, gsz, NT, XW], F16,
                               tag="xta" if gi == 0 else "xtbc")
                for bi in range(gsz):
                    b = b0 + bi
                    if b == 0:
                        src0 = _custom_ap(x[:], [(XROW, 128), (1, XW)])
                        nc.gpsimd.dma_start(xt[:, 0, 0, :], src0)
                        src1 = _custom_ap(
                            x[:], [(XROW, 128), (XW, NT - 1), (1, XW)],
                            extra_offset_elems=XW,
                        )
                        nc.gpsimd.dma_start(xt[:, 0, 1:NT, :], src1)
                    else:
                        src = _custom_ap(
                            x[:], [(XROW, 128), (1, NT * XW)],
                            extra_offset_elems=b * NT * XW,
                        )
                        nc.gpsimd.dma_start(
                            xt[:, bi, :, :], src
                        )
                    sample_xt[b] = (xt, bi)

            # Per-sample mode mix: 'g1' = 6 PE passes + 5 DVE builds,
            # 'direct1' = 7 passes + 4 builds. Sample 0 special-cased.
            MODES = [None, 'g1', 'g1', 'g1', 'g1', 'g1', 'g1',
                     'direct1']

            def pass_list(mode):
                if mode == 'direct':
                    ps = [(0, ('x', HALO))]
                    for k in range(1, MAXD + 1):
                        ps += [(k, ('x', HALO - k)), (k, ('x', HALO + k))]
                elif mode == 'direct1':
                    ps = [(0, ('x', HALO)),
                          (1, ('x', HALO - 1)), (1, ('x', HALO + 1))]
                    ps += [(k, ('av', k)) for k in range(2, MAXD + 1)]
                else:
                    ps = [(0, ('x', HALO))]
                    ps += [(k, ('av', k)) for k in range(1, MAXD + 1)]
                return ps

            for b in range(BPC):
                xt, bq = sample_xt[b]
                mode = MODES[b]
                build_ks = (tuple(range(1, MAXD + 1)) if mode in (None, 'g1')
                            else tuple(range(2, MAXD + 1)))

                # ---- av builds on DVE (fp16 2x mode) ----
                avs = {}
                for k in build_ks:
                    avs[k] = avpool.tile([128, NT, AVW], F16, tag=f"av{k}",
                                         name=f"av{k}")
                if b == 0:
                    # tile 0 is computed direct; build only tiles 1-4
                    for k in build_ks:
                        a0, a1, wdt, _ = AV_SPEC[k]
                        nc.vector.tensor_add(
                            avs[k][0:CTR, 1:NT, 0:wdt],
                            xt[0:CTR, bq, 1:NT, a0 : a0 + wdt],
                            xt[0:CTR, bq, 1:NT, a1 : a1 + wdt],
                        )
                else:
                    for k in build_ks:
                        a0, a1, wdt, _ = AV_SPEC[k]
                        nc.vector.tensor_add(
                            avs[k][0:CTR, :, 0:wdt],
                            xt[0:CTR, bq, :, a0 : a0 + wdt],
                            xt[0:CTR, bq, :, a1 : a1 + wdt],
                        )

                pts = [pspool.tile([128, W], f32, tag="pt", name=f"pt{t}")
                       for t in range(NT)]

                def moving(src, t):
                    kind, v = src
                    if kind == 'x':
                        return xt[0:CTR, bq, t, v : v + W]
                    _, _, _, mo = AV_SPEC[v]
                    return avs[v][0:CTR, t, mo : mo + W]

                if b == 0:
                    tile_passes = {0: pass_list('direct')}
                    for t in range(1, NT):
                        tile_passes[t] = pass_list('g1')
                    loop = [(t, pi) for t in range(NT)
                            for pi in range(len(tile_passes[t]))]
                else:
                    ps = pass_list(mode)
                    tile_passes = {t: ps for t in range(NT)}
                    if b == BPC - 1:
                        # t-outer: staggers psum completion so eviction +
                        # output drain inside the stream
                        loop = [(t, pi) for t in range(NT)
                                for pi in range(len(ps))]
                    else:
                        loop = [(t, pi) for pi in range(len(ps))
                                for t in range(NT)]
                for t, pi in loop:
                    j, src = tile_passes[t][pi]
                    nc.tensor.matmul(
                        pts[t][0:108, :],
                        wtile[0:CTR, j * 128 : j * 128 + 108],
                        moving(src, t),
                        start=(pi == 0),
                        stop=(pi == len(tile_passes[t]) - 1),
                    )

                # ---- eviction on ScalarE: psum -> fp16 SBUF copy ----
                otb = opool.tile([128, NT, W], F16, tag="otb")
                for t in range(NT):
                    nc.scalar.copy(otb[0:108, t, :], pts[t][0:108, :])

                # ---- output DMAs: fp16, 5.1KB runs per partition ----
                if b == BPC - 1:
                    splits = ((0, 1), (1, 2), (2, 3), (3, 4), (4, 5))
                elif b == BPC - 2:
                    splits = ((0, 2), (2, 4), (4, 5))
                else:
                    splits = ((0, NT),)
                for lo, hi in splits:
                    dstp = _custom_ap(
                        y[:], [(YROW, 128), (1, (hi - lo) * W)],
                        extra_offset_elems=(b * NT + lo) * W,
                    )
                    nc.gpsimd.dma_start(
                        dstp, otb[:, lo:hi, :]
                    )
    nc.compile()
    return nc


def _get_program():
    global _CACHED_NC
    if _CACHED_NC is None:
        _CACHED_NC = _build_program()
    return _CACHED_NC


def _pack_input(grid_spikes: np.ndarray) -> np.ndarray:
    """[B,H,W] f32 -> [128, B, NT, XW] fp16 with rows (103t+p-5)%512 and
    5 circular halo columns on each side."""
    p = np.arange(128)
    t = np.arange(NT)
    rows = (TR * t[:, None] + p[None, :] - HALO) % H  # [NT, 128]
    g = grid_spikes[:, rows, :]  # [B, NT, 128, W]
    g = np.concatenate([g[..., H - HALO :], g, g[..., :HALO]], axis=-1)
    return np.ascontiguousarray(
        g.transpose(2, 0, 1, 3).astype(np.float16)
    )  # [128, B, NT, XW]


def _run(grid_spikes, distance_weights, trace=False):
    grid_spikes = np.ascontiguousarray(np.asarray(grid_spikes, dtype=np.float32))
    distance_weights = np.asarray(distance_weights, dtype=np.float32)
    assert grid_spikes.shape == (B, H, W), grid_spikes.shape
    wb_np = _build_band_weights(distance_weights)
    xh = _pack_input(grid_spikes)  # [128, B, NT, XW] fp16

    nc = _get_program()
    in_maps = [
        {
            "x": np.ascontiguousarray(
                xh[:, i * BPC : (i + 1) * BPC]
            ).reshape(128, BPC * NT * XW),
            "wb": wb_np,
        }
        for i in range(NCORES)
    ]
    res = run_bass_kernel_spmd(nc, in_maps, list(range(NCORES)), trace=trace)
    yh = np.concatenate(
        [res.results[i]["y"].reshape(128, BPC, NT, W) for i in range(NCORES)],
        axis=1,
    )  # [128, B, NT, W] fp16
    # partition p of tile t holds out row 103t + p - 5 (valid p in 5..107);
    # rows 512..514 of the last tile are circular duplicates to drop.
    out = (
        yh[HALO : HALO + TR]
        .transpose(1, 2, 0, 3)
        .reshape(B, NT * TR, W)[:, :H, :]
        .astype(np.float32)
    )
    return np.ascontiguousarray(out), res


def kernel(grid_spikes, distance_weights):
    out, _ = _run(grid_spikes, distance_weights, trace=False)
    return out


def kernel_traced(grid_spikes, distance_weights):
    out, res = _run(grid_spikes, distance_weights, trace=True)
    return out, res
